# revision 1
# baseline (speedup 1.0000x reference)
"""Causal multi-head attention block (qkv -> attention -> proj) on 8 TRN2 cores.

Problem: x[2,2048,1024], w_qkv[3072,1024], b_qkv[3072], w_proj[1024,1024],
b_proj[1024]; H=16 heads, D=64; softmax scale 1/sqrt(1024).

Sharding: core = (batch b, head-group hg); 2 batches x 4 groups of 4 heads.
Each core computes qkv for its 4 heads, causal attention, and a partial
projection (its heads' columns of w_proj); host sums the 4 partials per batch
and adds b_proj.

Everything the PE contracts over lives partition-major: x is fed as xT[c,t];
weights are fed pre-transposed. The whole kernel is a single software
pipeline over t-chunks of 512: qkv(tc) -> attention(tc) -> proj(tc), so the
scalar engine's exp stream overlaps the tensor engine's qkv/proj matmuls.

Attention computes S^T[s,t] = k^T.T @ q^T directly (no transposes in the
S/P path), exp is applied unnormalized (scores are O(1) here), and V is
augmented with 64 ones-columns so the PV matmul yields the softmax
denominator replicated across partitions 64..127 -- normalization is then
one reciprocal_approx_fast + one DVE multiply per (head, chunk). Causality:
above-diagonal s-tiles are skipped; diagonal slabs are masked with
precomputed 0/1 masks. QK^T packs two heads in the PE via row tiling (K=64).
V is produced in [m,t] layout like q/k (wide N=512 matmuls) and moved to the
[t,m] layout PV needs via PE transposes of 128x128 blocks.

Attention-path tensors are fp16 (enables fast weight load, halves SBUF);
PSUM accumulation is always fp32; the projection runs in float32r.
"""

import numpy as np
from contextlib import ExitStack

import concourse.bass as bass
import concourse.bacc as bacc
import concourse.tile as tile
import concourse.mybir as mybir
from concourse.bass_utils import run_bass_kernel_spmd

B, T, C, H = 2, 2048, 1024, 16
D = C // H                  # 64, head dim
HPC = 4                     # heads per core
N_CORES = 8
NT = T // 128               # 16 t-tiles / s-tiles of 128
NCT = C // 128              # 8 contraction tiles over C
TCH = T // 512              # 4 t-chunks of 512
SCALE = 1.0 / np.sqrt(np.float32(C))   # 1/32

F32 = mybir.dt.float32
F32R = mybir.dt.float32r
F16 = mybir.dt.float16
EXP = mybir.ActivationFunctionType.Exp

VW = 2 * D                  # 128: per-head block in v_sb = [v_h (64) | ones (64)]

_CACHE = {}


def _build():
    """Build + compile the SPMD program (identical on all 8 cores)."""
    nc = bacc.Bacc("TRN2", target_bir_lowering=False, debug=False)

    xT = nc.dram_tensor("xT", [C, T], F16, kind="ExternalInput")          # x[b].T
    wqkvT = nc.dram_tensor("wqkvT", [C, 3 * HPC * D], F16, kind="ExternalInput")
    wpT = nc.dram_tensor("wpT", [HPC * D, C], F32R, kind="ExternalInput")
    bqkv = nc.dram_tensor("bqkv", [128, 6], F32, kind="ExternalInput")    # per m-tile
    ident = nc.dram_tensor("ident", [128, 128], F16, kind="ExternalInput")
    vones = nc.dram_tensor("vones", [128, NT * HPC * D], F16, kind="ExternalInput")
    mask = nc.dram_tensor("mask", [128, 2048], F16, kind="ExternalInput")  # 4x[128,512]
    y = nc.dram_tensor("y", [T, C], F32, kind="ExternalOutput")

    with tile.TileContext(nc) as tc, ExitStack() as ctx:
        sb = ctx.enter_context(tc.tile_pool(name="persist", bufs=1))

        # ---- persistent SBUF tensors ----
        wqkv_sb = sb.tile([128, NCT * 768], F16, tag="wqkv")       # [c-tile][m 768]
        wp_sb = sb.tile([128, 2 * C], F32R, tag="wp")              # [ci-tile][co 1024]
        bqkv_sb = sb.tile([128, 6], F32, tag="bqkv")
        ident_sb = sb.tile([128, 128], F16, tag="ident")
        mask_sb = sb.tile([128, 2048], F16, tag="mask")
        qk_sb = sb.tile([128, 6 * T], F16, tag="qk")   # q^T|k^T|v^T [m-tile][t]
        v_sb = sb.tile([128, NT * HPC * VW], F16, tag="v")  # [s-tile][h][v|ones]
        on_sb = sb.tile([128, 2 * T], F32R, tag="onorm")    # O_norm^T [ci-tile][t]

        nc.sync.dma_start(bqkv_sb[:], bqkv.ap())
        nc.sync.dma_start(ident_sb[:], ident.ap())
        for kt in range(2):
            nc.sync.dma_start(wp_sb[:, kt * C:(kt + 1) * C], wpT.ap()[kt * 128:(kt + 1) * 128, :])
        nc.sync.dma_start(mask_sb[:], mask.ap())
        # ones columns of v_sb (softmax denominator trick), cols 64..127/head
        vdst = v_sb[:].rearrange("p (s h e) -> p s h e", s=NT, h=HPC)[:, :, :, D:VW]
        vsrc = vones.ap().rearrange("p (s h e) -> p s h e", s=NT, h=HPC)
        nc.sync.dma_start(vdst, vsrc)

        # ---- fused pipeline: per t-chunk, qkv -> attention -> proj ----
        # PSUM budget (8 banks): sG [128,1024] x2 bufs = 4, acc0+acc1 = 2,
        # shared ps1 pool (qkv accum / v-transpose / proj out) x2 = 2.
        with tc.tile_pool(name="xTp", bufs=1) as xtp, \
             tc.tile_pool(name="ps1", bufs=2, space="PSUM") as ps1, \
             tc.tile_pool(name="ps2", bufs=2, space="PSUM") as ps2, \
             tc.tile_pool(name="psacc", bufs=1, space="PSUM") as psacc, \
             tc.tile_pool(name="att", bufs=4) as att, \
             tc.tile_pool(name="yst", bufs=4) as yst:
            xT_sb = xtp.tile([128, NCT * T], F16, tag="xT")       # [c-tile][t]
            for ct in range(NCT):
                nc.sync.dma_start(wqkv_sb[:, ct * 768:(ct + 1) * 768], wqkvT.ap()[ct * 128:(ct + 1) * 128, :])
                nc.sync.dma_start(xT_sb[:, ct * T:(ct + 1) * T], xT.ap()[ct * 128:(ct + 1) * 128, :])

            for tch in range(TCH):
                # qkv for this t-chunk: m-tiles 0,1=q 2,3=k 4,5=v (4 heads ea)
                for mt in range(6):
                    acc = ps1.tile([128, 512], F32, tag="qkacc")
                    for ct in range(NCT):
                        nc.tensor.matmul(
                            acc[:],
                            wqkv_sb[:, ct * 768 + mt * 128: ct * 768 + (mt + 1) * 128],
                            xT_sb[:, ct * T + tch * 512: ct * T + tch * 512 + 512],
                            start=(ct == 0), stop=(ct == NCT - 1),
                        )
                    nc.vector.tensor_scalar_add(
                        qk_sb[:, mt * T + tch * 512: mt * T + tch * 512 + 512],
                        acc[:], bqkv_sb[:, mt:mt + 1],
                    )
                # v -> [t, m] layout: PE transposes of 128x128 (2 heads/blk)
                for st in range(4 * tch, 4 * tch + 4):
                    for hv in range(2):
                        tp = ps1.tile([128, 512], F32, tag="qkacc")
                        tp16 = tp[:].bitcast(F16)[:, 0:128]
                        nc.tensor.transpose(
                            tp16,
                            qk_sb[:, (4 + hv) * T + st * 128: (4 + hv) * T + st * 128 + 128],
                            ident_sb[:])
                        dst = v_sb[:, st * HPC * VW + 2 * hv * VW: st * HPC * VW + (2 * hv + 2) * VW].rearrange(
                            "p (h e) -> p h e", h=2)[:, :, 0:D]
                        src = tp16.rearrange("p (h d) -> p h d", h=2)
                        nc.vector.tensor_copy(dst, src)

                # attention for this t-chunk
                for hp in range(2):      # head pair (heads 2hp, 2hp+1)
                    qoff = hp * T        # q m-tile = hp
                    koff = (2 + hp) * T  # k m-tile = 2+hp
                    acc0 = psacc.tile([128, 512], F32, tag="acc0")
                    acc1 = psacc.tile([128, 512], F32, tag="acc1")
                    n_slab = 2 * (tch + 1)
                    for g in range(n_slab):
                        sG0 = ps2.tile([128, 1024], F32, tag="sG")
                        sG1 = ps2.tile([128, 1024], F32, tag="sG")
                        for j in range(2):
                            st = 2 * g + j
                            nc.tensor.matmul(
                                sG0[:, j * 512:(j + 1) * 512],
                                qk_sb[0:64, koff + st * 128: koff + st * 128 + 128],
                                qk_sb[0:64, qoff + tch * 512: qoff + tch * 512 + 512],
                                start=True, stop=True, tile_position=(0, 0),
                            )
                            nc.tensor.matmul(
                                sG1[:, j * 512:(j + 1) * 512],
                                qk_sb[64:128, koff + st * 128: koff + st * 128 + 128],
                                qk_sb[64:128, qoff + tch * 512: qoff + tch * 512 + 512],
                                start=True, stop=True, tile_position=(64, 0),
                            )
                        p0 = att.tile([128, 1024], F16, tag="p0")
                        p1 = att.tile([128, 1024], F16, tag="p1")
                        nc.scalar.activation(p0[:], sG0[:], EXP, scale=float(SCALE))
                        nc.scalar.activation(p1[:], sG1[:], EXP, scale=float(SCALE))
                        if g >= 2 * tch:   # diagonal slab: causal 0/1 mask
                            mi = (g - 2 * tch) * 1024
                            m = mask_sb[:, mi:mi + 1024]
                            nc.vector.tensor_mul(p0[:], p0[:], m)
                            nc.vector.tensor_mul(p1[:], p1[:], m)
                        first, last = (g == 0), (g == n_slab - 1)
                        for j in range(2):
                            st = 2 * g + j
                            nc.tensor.matmul(
                                acc0[:],
                                v_sb[:, st * HPC * VW + (2 * hp) * VW: st * HPC * VW + (2 * hp) * VW + VW],
                                p0[:, j * 512:(j + 1) * 512],
                                start=(first and j == 0), stop=(last and j == 1),
                            )
                            nc.tensor.matmul(
                                acc1[:],
                                v_sb[:, st * HPC * VW + (2 * hp + 1) * VW: st * HPC * VW + (2 * hp + 1) * VW + VW],
                                p1[:, j * 512:(j + 1) * 512],
                                start=(first and j == 0), stop=(last and j == 1),
                            )
                    # normalize: O_norm^T = O^T*(1/l), l on rows 64..127
                    for i, acc in ((0, acc0), (1, acc1)):
                        a = 2 * hp + i   # head index in core
                        # full-tile recip: the custom-DVE op mishandles
                        # partition slices; rows 0..63 are garbage, unused
                        rl = att.tile([128, 512], F32, tag="rl")
                        nc.vector.reciprocal_approx_fast(rl[:], acc[:])
                        po = (a % 2) * 64
                        dst = on_sb[po:po + 64,
                                    (a // 2) * T + tch * 512:(a // 2) * T + tch * 512 + 512]
                        nc.vector.tensor_mul(dst, acc[0:D, :], rl[64:128, :])

                # proj for this t-chunk (needs all 4 heads at these t)
                for tt in range(4 * tch, 4 * tch + 4):
                    for cc in range(2):
                        acc = ps1.tile([128, 512], F32, tag="qkacc")
                        for kt in range(2):
                            nc.tensor.matmul(
                                acc[:],
                                on_sb[:, kt * T + tt * 128: kt * T + tt * 128 + 128],
                                wp_sb[:, kt * C + cc * 512: kt * C + cc * 512 + 512],
                                start=(kt == 0), stop=(kt == 1),
                            )
                        ytile = yst.tile([128, 512], F32, tag="ytile")
                        nc.vector.tensor_copy(ytile[:], acc[:])
                        nc.sync.dma_start(
                            y.ap()[tt * 128:(tt + 1) * 128, cc * 512:(cc + 1) * 512],
                            ytile[:],
                        )

    nc.compile()
    return nc


def _causal_masks():
    """mask[p, r*512 + j] = 1.0 if (128*r + p) <= j else 0.0, r in 0..3."""
    p = np.arange(128)[:, None]
    j = np.arange(512)[None, :]
    cols = [((128 * r + p) <= j).astype(np.float32) for r in range(4)]
    return np.concatenate(cols, axis=1)


def _in_maps(x, w_qkv, b_qkv, w_proj):
    mask = _causal_masks()
    vones = np.ones((128, NT * HPC * D), dtype=np.float32)
    maps = []
    for core in range(N_CORES):
        b, hg = divmod(core, 4)
        h0 = hg * HPC                       # first global head of this core
        r0 = h0 * D                         # first q row
        q_w = w_qkv[r0:r0 + HPC * D]                    # [256, C]
        k_w = w_qkv[C + r0:C + r0 + HPC * D]
        v_w = w_qkv[2 * C + r0:2 * C + r0 + HPC * D]
        wqkvT = np.ascontiguousarray(np.concatenate([q_w, k_w, v_w], axis=0).T)
        wpT = np.ascontiguousarray(w_proj[:, r0:r0 + HPC * D].T)    # [256, C]
        bqkv = np.ascontiguousarray(np.concatenate(
            [b_qkv[r0:r0 + HPC * D], b_qkv[C + r0:C + r0 + HPC * D],
             b_qkv[2 * C + r0:2 * C + r0 + HPC * D]]).reshape(6, 128).T)  # [128,6]
        maps.append({
            "xT": np.ascontiguousarray(x[b].T).astype(np.float16),
            "wqkvT": wqkvT.astype(np.float16),
            "wpT": wpT,
            "bqkv": bqkv,
            "ident": np.eye(128, dtype=np.float16),
            "vones": vones.astype(np.float16),
            "mask": mask.astype(np.float16),
        })
    return maps


def kernel(x, w_qkv, b_qkv, w_proj, b_proj, _trace=False, _tmpdir=None):
    x = np.asarray(x, dtype=np.float32)
    w_qkv = np.asarray(w_qkv, dtype=np.float32)
    b_qkv = np.asarray(b_qkv, dtype=np.float32)
    w_proj = np.asarray(w_proj, dtype=np.float32)
    b_proj = np.asarray(b_proj, dtype=np.float32)

    if "nc" not in _CACHE:
        _CACHE["nc"] = _build()
    nc = _CACHE["nc"]

    maps = _in_maps(x, w_qkv, b_qkv, w_proj)
    kw = {}
    if _trace:
        kw = {"trace": True, "tmpdir": _tmpdir}
    res = run_bass_kernel_spmd(nc, maps, list(range(N_CORES)), **kw)

    out = np.empty((B, T, C), dtype=np.float32)
    for b in range(B):
        acc = res.results[4 * b]["y"].astype(np.float32)
        for hg in range(1, 4):
            acc = acc + res.results[4 * b + hg]["y"]
        out[b] = acc + b_proj[None, :]
    if _trace:
        return out, res
    return out



# revision 4
# speedup vs baseline: 1.2344x; 1.2344x over previous
"""Causal multi-head attention block (qkv -> attention -> proj) on 8 TRN2 cores.

Problem: x[2,2048,1024], w_qkv[3072,1024], b_qkv[3072], w_proj[1024,1024],
b_proj[1024]; H=16 heads, D=64; softmax scale 1/sqrt(1024).

Sharding: core = (batch b, head-group hg); 2 batches x 4 groups of 4 heads.
Each core computes qkv for its 4 heads, causal attention, and a partial
projection (its heads' columns of w_proj); host sums the 4 partials per batch
and adds the folded bias (w_proj @ b_v + b_proj).

The whole kernel is ONE software-pipelined instruction stream built around
the scalar engine's exp throughput: attention slabs (QK -> exp -> PV) are the
backbone, and qkv / proj matmuls of other t-chunks are interleaved between a
slab's exp and its PV so the tensor engine never stalls waiting on the
scalar engine (which also keeps the PE at its top p-state).

Layouts: everything the PE contracts over is partition-major. q,k are
produced [dims, t] (weights stationary); v is produced directly in [t, dims]
(x tiles stationary) so no PE transposes are needed; the PV stationary
v-tile is [v_h (64) | ones (64)] so the PV matmul yields the softmax
denominator on partitions 64..127 for free. Causality: above-diagonal
s-slabs are skipped; diagonal slabs are masked with precomputed 0/1 masks on
the (otherwise idle) gpsimd engine. QK packs two heads into the PE via row
tiling (K=64), and each slab's scores for both heads live in one
[128,2048] PSUM tile so a single activation instruction exps them all.

All attention-path and projection tensors are fp16 (PSUM accumulation fp32);
the per-core partial y is returned fp16 and summed on the host in fp32.
"""

import numpy as np
from contextlib import ExitStack

import concourse.bass as bass
import concourse.bacc as bacc
import concourse.tile as tile
import concourse.mybir as mybir
from concourse.bass_utils import run_bass_kernel_spmd

B, T, C, H = 2, 2048, 1024, 16
D = C // H                  # 64, head dim
HPC = 4                     # heads per core
N_CORES = 8
NT = T // 128               # 16 t-tiles / s-tiles of 128
NCT = C // 128              # 8 contraction tiles over C
TCH = T // 512              # 4 t-chunks of 512
SCALE = 1.0 / np.sqrt(np.float32(C))   # 1/32

F32 = mybir.dt.float32
F16 = mybir.dt.float16
EXP = mybir.ActivationFunctionType.Exp

VW = 2 * D                  # 128: per-head block in v_sb = [v_h (64) | ones (64)]
MM_NS = 230.0               # planning est: one N=512 fp16 matmul slot

_CACHE = {}


def _build():
    """Build + compile the SPMD program (identical on all 8 cores)."""
    nc = bacc.Bacc("TRN2", target_bir_lowering=False, debug=False)

    xT = nc.dram_tensor("xT", [C, T], F16, kind="ExternalInput")          # x[b].T
    wqkvT = nc.dram_tensor("wqkvT", [C, 3 * HPC * D], F16, kind="ExternalInput")
    wpT = nc.dram_tensor("wpT", [HPC * D, C], F16, kind="ExternalInput")
    bqkv = nc.dram_tensor("bqkv", [128, 4], F32, kind="ExternalInput")    # q,k m-tiles
    vones = nc.dram_tensor("vones", [128, NT * HPC * D], F16, kind="ExternalInput")
    mask = nc.dram_tensor("mask", [128, 2048], F16, kind="ExternalInput")  # 4x[128,512]
    y = nc.dram_tensor("y", [T, C], F16, kind="ExternalOutput")

    with tile.TileContext(nc) as tc, ExitStack() as ctx:
        sb = ctx.enter_context(tc.tile_pool(name="persist", bufs=1))

        # ---- persistent SBUF tensors ----
        wqkv_sb = sb.tile([128, NCT * 768], F16, tag="wqkv")       # [c-tile][m 768]
        wp_sb = sb.tile([128, 2 * C], F16, tag="wp")               # [ci-tile][co 1024]
        bqkv_sb = sb.tile([128, 4], F32, tag="bqkv")
        mask_sb = sb.tile([128, 2048], F16, tag="mask")
        x_sb = sb.tile([128, NCT * T], F16, tag="x")               # [c-tile][t]
        qk_sb = sb.tile([128, 4 * T], F16, tag="qk")   # q^T|k^T [m-tile][t]
        v_sb = sb.tile([128, NT * HPC * VW], F16, tag="v")  # [s-tile][h][v|ones]
        on_sb = sb.tile([128, 2 * T], F16, tag="onorm")     # O_norm^T [ci-tile][t]

        with tc.tile_pool(name="psg", bufs=1, space="PSUM") as psg, \
             tc.tile_pool(name="psacc", bufs=1, space="PSUM") as psacc, \
             tc.tile_pool(name="gem", bufs=2, space="PSUM") as gem, \
             tc.tile_pool(name="att", bufs=4) as att, \
             tc.tile_pool(name="rlp", bufs=2) as rlp, \
             tc.tile_pool(name="yst", bufs=4) as yst:

            # ---- input DMAs (gpsimd queue: cheap dispatch, overlaps all) ----
            nc.gpsimd.dma_start(bqkv_sb[:], bqkv.ap())
            nc.gpsimd.dma_start(mask_sb[:], mask.ap())
            for ct in range(NCT):       # chunk-0 x + weights first (prologue)
                nc.gpsimd.dma_start(wqkv_sb[:, ct * 768:(ct + 1) * 768],
                                    wqkvT.ap()[ct * 128:(ct + 1) * 128, :])
                nc.gpsimd.dma_start(x_sb[:, ct * T: ct * T + 512],
                                    xT.ap()[ct * 128:(ct + 1) * 128, 0:512])
            # ones columns of v_sb (softmax denominator trick), cols 64..127/head
            vdst = v_sb[:].rearrange("p (s h e) -> p s h e", s=NT, h=HPC)[:, :, :, D:VW]
            vsrc = vones.ap().rearrange("p (s h e) -> p s h e", s=NT, h=HPC)
            nc.gpsimd.dma_start(vdst, vsrc)
            for ct in range(NCT):       # x chunks 1..3
                nc.gpsimd.dma_start(x_sb[:, ct * T + 512:(ct + 1) * T],
                                    xT.ap()[ct * 128:(ct + 1) * 128, 512:T])
            for kt in range(2):
                nc.gpsimd.dma_start(wp_sb[:, kt * C:(kt + 1) * C],
                                    wpT.ap()[kt * 128:(kt + 1) * 128, :])

            # ---- filler work units (generators; yield ~est ns per PE slot) ----
            def qk_gemm_unit(tch, mt):
                """q or k m-tile GEMM for one t-chunk + bias move to qk_sb."""
                acc = gem.tile([128, 512], F32, tag="gacc", name="gacc")
                for ct in range(NCT):
                    nc.tensor.matmul(
                        acc[:],
                        wqkv_sb[:, ct * 768 + mt * 128: ct * 768 + (mt + 1) * 128],
                        x_sb[:, ct * T + tch * 512: ct * T + tch * 512 + 512],
                        start=(ct == 0), stop=(ct == NCT - 1),
                    )
                    yield MM_NS
                nc.vector.tensor_scalar_add(
                    qk_sb[:, mt * T + tch * 512: mt * T + tch * 512 + 512],
                    acc[:], bqkv_sb[:, mt:mt + 1],
                )

            def v_gemm_unit(tch, i):
                """v for t-tile 4*tch+i, produced directly in [t, m] layout."""
                tt = 4 * tch + i
                acc = gem.tile([128, 512], F32, tag="gacc", name="vacc")
                for ct in range(NCT):
                    nc.tensor.matmul(
                        acc[:, 0:256],
                        x_sb[:, ct * T + tt * 128: ct * T + tt * 128 + 128],
                        wqkv_sb[:, ct * 768 + 512: ct * 768 + 768],
                        start=(ct == 0), stop=(ct == NCT - 1),
                    )
                    yield MM_NS / 2
                dst = v_sb[:, tt * HPC * VW:(tt + 1) * HPC * VW].rearrange(
                    "p (h e) -> p h e", h=HPC)[:, :, 0:D]
                src = acc[:, 0:256].rearrange("p (h d) -> p h d", h=HPC)
                nc.vector.tensor_copy(dst, src)   # gpsimd can't read PSUM

            def proj_unit(tch, i):
                """proj for t-tile 4*tch+i (all 4 heads), y write-out."""
                tt = 4 * tch + i
                yt = yst.tile([128, C], F16, tag="ytile", name="yt")
                for cc in range(2):
                    acc = gem.tile([128, 512], F32, tag="gacc", name="pacc")
                    for kt in range(2):
                        nc.tensor.matmul(
                            acc[:],
                            on_sb[:, kt * T + tt * 128: kt * T + tt * 128 + 128],
                            wp_sb[:, kt * C + cc * 512: kt * C + cc * 512 + 512],
                            start=(kt == 0), stop=(kt == 1),
                        )
                        yield MM_NS
                    nc.vector.tensor_copy(yt[:, cc * 512:(cc + 1) * 512], acc[:])
                nc.sync.dma_start(y.ap()[tt * 128:(tt + 1) * 128, :], yt[:])

            # FIFO of filler units; per chunk: hp0's k,q first, then v, then hp1
            fifo = []
            for tch in range(TCH):
                fifo.append((('k', tch, 0), qk_gemm_unit(tch, 2)))
                fifo.append((('q', tch, 0), qk_gemm_unit(tch, 0)))
                for i in range(4):
                    fifo.append((('v', tch, i), v_gemm_unit(tch, i)))
                fifo.append((('k', tch, 1), qk_gemm_unit(tch, 3)))
                fifo.append((('q', tch, 1), qk_gemm_unit(tch, 1)))

            pos = [0]
            done = set()

            def step_front():
                """Advance the head unit by one PE slot; returns est ns."""
                tag, gen = fifo[pos[0]]
                try:
                    return next(gen)
                except StopIteration:
                    done.add(tag)
                    pos[0] += 1
                    return 0.0

            def pull(ns):
                spent = 0.0
                while spent < ns and pos[0] < len(fifo):
                    spent += step_front()

            def drain(tags):
                while not tags.issubset(done) and pos[0] < len(fifo):
                    step_front()

            # ---- attention backbone: slabs of 2 s-tiles x 512 t, 2 heads ----
            def attention(tch, hp):
                qoff = hp * T        # q m-tile = hp
                koff = (2 + hp) * T  # k m-tile = 2+hp
                acc0 = psacc.tile([128, 512], F32, tag="acc0", name="acc0")
                acc1 = psacc.tile([128, 512], F32, tag="acc1", name="acc1")
                n_slab = 2 * (tch + 1)
                for g in range(n_slab):
                    sg = psg.tile([128, 2048], F32, tag="sG", name="sg")
                    p = att.tile([128, 2048], F16, tag="p", name="p")
                    for j in range(2):
                        st = 2 * g + j
                        nc.tensor.matmul(
                            sg[:, j * 512:(j + 1) * 512],
                            qk_sb[0:64, koff + st * 128: koff + st * 128 + 128],
                            qk_sb[0:64, qoff + tch * 512: qoff + tch * 512 + 512],
                            start=True, stop=True, tile_position=(0, 0),
                        )
                        nc.tensor.matmul(
                            sg[:, 1024 + j * 512: 1024 + (j + 1) * 512],
                            qk_sb[64:128, koff + st * 128: koff + st * 128 + 128],
                            qk_sb[64:128, qoff + tch * 512: qoff + tch * 512 + 512],
                            start=True, stop=True, tile_position=(64, 0),
                        )
                    nc.scalar.activation(p[:], sg[:], EXP, scale=float(SCALE))
                    if g >= 2 * tch:   # diagonal slab: causal 0/1 mask (gpsimd)
                        mi = (g - 2 * tch) * 1024
                        m = mask_sb[:, mi:mi + 1024]
                        nc.gpsimd.tensor_mul(p[:, 0:1024], p[:, 0:1024], m)
                        nc.gpsimd.tensor_mul(p[:, 1024:2048], p[:, 1024:2048], m)
                    # filler matmuls cover the exp latency before PV needs p
                    pull(1150.0)
                    first, last = (g == 0), (g == n_slab - 1)
                    for j in range(2):
                        st = 2 * g + j
                        nc.tensor.matmul(
                            acc0[:],
                            v_sb[:, st * HPC * VW + (2 * hp) * VW:
                                 st * HPC * VW + (2 * hp) * VW + VW],
                            p[:, j * 512:(j + 1) * 512],
                            start=(first and j == 0), stop=(last and j == 1),
                        )
                        nc.tensor.matmul(
                            acc1[:],
                            v_sb[:, st * HPC * VW + (2 * hp + 1) * VW:
                                 st * HPC * VW + (2 * hp + 1) * VW + VW],
                            p[:, 1024 + j * 512: 1024 + (j + 1) * 512],
                            start=(first and j == 0), stop=(last and j == 1),
                        )
                # normalize: O_norm^T = O^T*(1/l), l on rows 64..127
                for i, acc in ((0, acc0), (1, acc1)):
                    a = 2 * hp + i   # head index in core
                    # full-tile recip: the custom-DVE op mishandles
                    # partition slices; rows 0..63 are garbage, unused
                    rl = rlp.tile([128, 512], F32, tag="rl", name="rl")
                    nc.vector.reciprocal_approx_fast(rl[:], acc[:])
                    po = (a % 2) * 64
                    dst = on_sb[po:po + 64,
                                (a // 2) * T + tch * 512:(a // 2) * T + tch * 512 + 512]
                    nc.vector.tensor_mul(dst, acc[0:D, :], rl[64:128, :])

            # ---- driver: attention phases with qkv/proj interleaved ----
            for tch in range(TCH):
                for hp in range(2):
                    req = set()
                    for c in range(tch + 1):
                        req |= {('k', c, hp), ('q', c, hp)}
                        req |= {('v', c, i) for i in range(4)}
                    drain(req)
                    attention(tch, hp)
                for i in range(4):
                    fifo.append((('c', tch, i), proj_unit(tch, i)))
            while pos[0] < len(fifo):
                step_front()

    nc.compile()
    return nc


def _causal_masks():
    """mask[p, r*512 + j] = 1.0 if (128*r + p) <= j else 0.0, r in 0..3."""
    p = np.arange(128)[:, None]
    j = np.arange(512)[None, :]
    cols = [((128 * r + p) <= j).astype(np.float32) for r in range(4)]
    return np.concatenate(cols, axis=1)


def _in_maps(x, w_qkv, b_qkv, w_proj):
    mask = _causal_masks()
    vones = np.ones((128, NT * HPC * D), dtype=np.float32)
    maps = []
    for core in range(N_CORES):
        b, hg = divmod(core, 4)
        h0 = hg * HPC                       # first global head of this core
        r0 = h0 * D                         # first q row
        q_w = w_qkv[r0:r0 + HPC * D]                    # [256, C]
        k_w = w_qkv[C + r0:C + r0 + HPC * D]
        v_w = w_qkv[2 * C + r0:2 * C + r0 + HPC * D]
        wqkvT = np.ascontiguousarray(np.concatenate([q_w, k_w, v_w], axis=0).T)
        wpT = np.ascontiguousarray(w_proj[:, r0:r0 + HPC * D].T)    # [256, C]
        bqkv = np.ascontiguousarray(np.concatenate(
            [b_qkv[r0:r0 + HPC * D], b_qkv[C + r0:C + r0 + HPC * D]]
        ).reshape(4, 128).T)                                         # [128,4]
        maps.append({
            "xT": np.ascontiguousarray(x[b].T).astype(np.float16),
            "wqkvT": wqkvT.astype(np.float16),
            "wpT": wpT.astype(np.float16),
            "bqkv": bqkv,
            "vones": vones.astype(np.float16),
            "mask": mask.astype(np.float16),
        })
    return maps


def kernel(x, w_qkv, b_qkv, w_proj, b_proj, _trace=False, _tmpdir=None):
    x = np.asarray(x, dtype=np.float32)
    w_qkv = np.asarray(w_qkv, dtype=np.float32)
    b_qkv = np.asarray(b_qkv, dtype=np.float32)
    w_proj = np.asarray(w_proj, dtype=np.float32)
    b_proj = np.asarray(b_proj, dtype=np.float32)

    if "nc" not in _CACHE:
        _CACHE["nc"] = _build()
    nc = _CACHE["nc"]

    maps = _in_maps(x, w_qkv, b_qkv, w_proj)
    kw = {}
    if _trace:
        kw = {"trace": True, "tmpdir": _tmpdir}
    res = run_bass_kernel_spmd(nc, maps, list(range(N_CORES)), **kw)

    # v-bias flows linearly through attention: fold w_proj @ b_v into the
    # output bias added on the host.
    b_eff = w_proj @ b_qkv[2 * C:3 * C] + b_proj
    out = np.empty((B, T, C), dtype=np.float32)
    for b in range(B):
        acc = res.results[4 * b]["y"].astype(np.float32)
        for hg in range(1, 4):
            acc = acc + res.results[4 * b + hg]["y"].astype(np.float32)
        out[b] = acc + b_eff[None, :]
    if _trace:
        return out, res
    return out


# revision 6
# speedup vs baseline: 1.3375x; 1.0836x over previous
"""Causal multi-head attention block (qkv -> attention -> proj) on 8 TRN2 cores.

Problem: x[2,2048,1024], w_qkv[3072,1024], b_qkv[3072], w_proj[1024,1024],
b_proj[1024]; H=16 heads, D=64; softmax scale 1/sqrt(1024).

Sharding: core = (batch b, head-group hg); 2 batches x 4 groups of 4 heads.
Each core computes qkv for its 4 heads, causal attention, and a partial
projection (its heads' columns of w_proj); host sums the 4 partials per batch
and adds the folded bias (w_proj @ b_v + b_proj).

The whole kernel is ONE software-pipelined instruction stream built around
the scalar engine's exp throughput: attention slabs (QK -> exp -> PV) are the
backbone, and qkv / proj matmuls of other t-chunks are interleaved between a
slab's exp and its PV so the tensor engine never stalls waiting on the
scalar engine (which also keeps the PE at its top p-state).

Layouts: everything the PE contracts over is partition-major. q,k are
produced [dims, t] (weights stationary); v is produced directly in [t, dims]
(x tiles stationary) so no PE transposes are needed; the PV stationary
v-tile is [v_h (64) | ones (64)] so the PV matmul yields the softmax
denominator on partitions 64..127 for free. Causality: above-diagonal
s-slabs are skipped; diagonal slabs are masked with precomputed 0/1 masks on
the (otherwise idle) gpsimd engine. QK packs two heads into the PE via row
tiling (K=64), and each slab's scores for both heads live in one
[128,2048] PSUM tile so a single activation instruction exps them all.

All attention-path and projection tensors are fp16 (PSUM accumulation fp32);
the per-core partial y is returned fp16 and summed on the host in fp32.
"""

import numpy as np
from contextlib import ExitStack

import concourse.bass as bass
import concourse.bacc as bacc
import concourse.tile as tile
import concourse.mybir as mybir
from concourse.bass_utils import run_bass_kernel_spmd

B, T, C, H = 2, 2048, 1024, 16
D = C // H                  # 64, head dim
HPC = 4                     # heads per core
N_CORES = 8
NT = T // 128               # 16 t-tiles / s-tiles of 128
NCT = C // 128              # 8 contraction tiles over C
TCH = T // 512              # 4 t-chunks of 512
SCALE = 1.0 / np.sqrt(np.float32(C))   # 1/32

F32 = mybir.dt.float32
F16 = mybir.dt.float16
EXP = mybir.ActivationFunctionType.Exp

VW = 2 * D                  # 128: per-head block in v_sb = [v_h (64) | ones (64)]
MM_NS = 230.0               # planning est: one N=512 fp16 matmul slot

_CACHE = {}


def _build():
    """Build + compile the SPMD program (identical on all 8 cores)."""
    nc = bacc.Bacc("TRN2", target_bir_lowering=False, debug=False)

    xT = nc.dram_tensor("xT", [C, T], F16, kind="ExternalInput")          # x[b].T
    wqkvT = nc.dram_tensor("wqkvT", [C, 3 * HPC * D], F16, kind="ExternalInput")
    wpT = nc.dram_tensor("wpT", [HPC * D, C], F16, kind="ExternalInput")
    bqkv = nc.dram_tensor("bqkv", [128, 4], F32, kind="ExternalInput")    # q,k m-tiles
    vones = nc.dram_tensor("vones", [128, NT * HPC * D], F16, kind="ExternalInput")
    mask = nc.dram_tensor("mask", [128, 2048], F16, kind="ExternalInput")  # 4x[128,512]
    y = nc.dram_tensor("y", [T, C], F16, kind="ExternalOutput")

    with tile.TileContext(nc) as tc, ExitStack() as ctx:
        sb = ctx.enter_context(tc.tile_pool(name="persist", bufs=1))

        # ---- persistent SBUF tensors ----
        wqkv_sb = sb.tile([128, NCT * 768], F16, tag="wqkv")       # [c-tile][m 768]
        wp_sb = sb.tile([128, 2 * C], F16, tag="wp")               # [ci-tile][co 1024]
        bqkv_sb = sb.tile([128, 4], F32, tag="bqkv")
        mask_sb = sb.tile([128, 2048], F16, tag="mask")
        x_sb = sb.tile([128, NCT * T], F16, tag="x")               # [c-tile][t]
        qk_sb = sb.tile([128, 4 * T], F16, tag="qk")   # q^T|k^T [m-tile][t]
        v_sb = sb.tile([128, NT * HPC * VW], F16, tag="v")  # [s-tile][h][v|ones]
        on_sb = sb.tile([128, 2 * T], F16, tag="onorm")     # O_norm^T [ci-tile][t]

        with tc.tile_pool(name="psg", bufs=1, space="PSUM") as psg, \
             tc.tile_pool(name="psacc", bufs=1, space="PSUM") as psacc, \
             tc.tile_pool(name="gem", bufs=2, space="PSUM") as gem, \
             tc.tile_pool(name="att", bufs=4) as att, \
             tc.tile_pool(name="rlp", bufs=2) as rlp, \
             tc.tile_pool(name="yst", bufs=4) as yst:

            # ---- input DMAs (gpsimd queue: cheap dispatch, overlaps all) ----
            nc.gpsimd.dma_start(bqkv_sb[:], bqkv.ap())
            nc.gpsimd.dma_start(mask_sb[:], mask.ap())
            for ct in range(NCT):       # chunk-0 x + weights first (prologue)
                nc.gpsimd.dma_start(wqkv_sb[:, ct * 768:(ct + 1) * 768],
                                    wqkvT.ap()[ct * 128:(ct + 1) * 128, :])
                nc.gpsimd.dma_start(x_sb[:, ct * T: ct * T + 512],
                                    xT.ap()[ct * 128:(ct + 1) * 128, 0:512])
            # ones columns of v_sb (softmax denominator trick), cols 64..127/head
            vdst = v_sb[:].rearrange("p (s h e) -> p s h e", s=NT, h=HPC)[:, :, :, D:VW]
            vsrc = vones.ap().rearrange("p (s h e) -> p s h e", s=NT, h=HPC)
            nc.gpsimd.dma_start(vdst, vsrc)
            for ct in range(NCT):       # x chunks 1..3
                nc.gpsimd.dma_start(x_sb[:, ct * T + 512:(ct + 1) * T],
                                    xT.ap()[ct * 128:(ct + 1) * 128, 512:T])
            for kt in range(2):
                nc.gpsimd.dma_start(wp_sb[:, kt * C:(kt + 1) * C],
                                    wpT.ap()[kt * 128:(kt + 1) * 128, :])

            # ---- filler work units (generators; yield ~est ns per PE slot) ----
            def qk_gemm_unit(tch, mt):
                """q or k m-tile GEMM for one t-chunk + bias move to qk_sb."""
                acc = gem.tile([128, 512], F32, tag="gacc", name="gacc")
                for ct in range(NCT):
                    nc.tensor.matmul(
                        acc[:],
                        wqkv_sb[:, ct * 768 + mt * 128: ct * 768 + (mt + 1) * 128],
                        x_sb[:, ct * T + tch * 512: ct * T + tch * 512 + 512],
                        start=(ct == 0), stop=(ct == NCT - 1),
                    )
                    yield MM_NS
                nc.vector.tensor_scalar_add(
                    qk_sb[:, mt * T + tch * 512: mt * T + tch * 512 + 512],
                    acc[:], bqkv_sb[:, mt:mt + 1],
                )

            def v_gemm_unit(tch, i):
                """v for t-tile 4*tch+i, produced directly in [t, m] layout."""
                tt = 4 * tch + i
                acc = gem.tile([128, 512], F32, tag="gacc", name="vacc")
                for ct in range(NCT):
                    nc.tensor.matmul(
                        acc[:, 0:256],
                        x_sb[:, ct * T + tt * 128: ct * T + tt * 128 + 128],
                        wqkv_sb[:, ct * 768 + 512: ct * 768 + 768],
                        start=(ct == 0), stop=(ct == NCT - 1),
                    )
                    yield MM_NS / 2
                dst = v_sb[:, tt * HPC * VW:(tt + 1) * HPC * VW].rearrange(
                    "p (h e) -> p h e", h=HPC)[:, :, 0:D]
                src = acc[:, 0:256].rearrange("p (h d) -> p h d", h=HPC)
                nc.vector.tensor_copy(dst, src)   # gpsimd can't read PSUM

            def proj_unit(tch, i):
                """proj for t-tile 4*tch+i (all 4 heads), y write-out."""
                tt = 4 * tch + i
                yt = yst.tile([128, C], F16, tag="ytile", name="yt")
                for cc in range(2):
                    acc = gem.tile([128, 512], F32, tag="gacc", name="pacc")
                    for kt in range(2):
                        nc.tensor.matmul(
                            acc[:],
                            on_sb[:, kt * T + tt * 128: kt * T + tt * 128 + 128],
                            wp_sb[:, kt * C + cc * 512: kt * C + cc * 512 + 512],
                            start=(kt == 0), stop=(kt == 1),
                        )
                        yield MM_NS
                    nc.vector.tensor_copy(yt[:, cc * 512:(cc + 1) * 512], acc[:])
                nc.sync.dma_start(y.ap()[tt * 128:(tt + 1) * 128, :], yt[:])

            # FIFO of filler units; per chunk: hp0's k,q first, then v, then hp1
            fifo = []
            for tch in range(TCH):
                fifo.append((('k', tch, 0), qk_gemm_unit(tch, 2)))
                fifo.append((('q', tch, 0), qk_gemm_unit(tch, 0)))
                for i in range(4):
                    fifo.append((('v', tch, i), v_gemm_unit(tch, i)))
                fifo.append((('k', tch, 1), qk_gemm_unit(tch, 3)))
                fifo.append((('q', tch, 1), qk_gemm_unit(tch, 1)))

            pos = [0]
            done = set()

            def step_front():
                """Advance the head unit by one PE slot; returns est ns."""
                tag, gen = fifo[pos[0]]
                try:
                    return next(gen)
                except StopIteration:
                    done.add(tag)
                    pos[0] += 1
                    return 0.0

            def pull(ns):
                spent = 0.0
                while spent < ns and pos[0] < len(fifo):
                    spent += step_front()

            def drain(tags):
                while not tags.issubset(done) and pos[0] < len(fifo):
                    step_front()

            # ---- attention backbone: slabs of 2 s-tiles x 512 t, 2 heads ----
            def attention(tch, hp, quota):
                qoff = hp * T        # q m-tile = hp
                koff = (2 + hp) * T  # k m-tile = 2+hp
                acc0 = psacc.tile([128, 512], F32, tag="acc0", name="acc0")
                acc1 = psacc.tile([128, 512], F32, tag="acc1", name="acc1")
                n_slab = 2 * (tch + 1)
                for g in range(n_slab):
                    sg = psg.tile([128, 2048], F32, tag="sG", name="sg")
                    p = att.tile([128, 2048], F16, tag="p", name="p")
                    diag = (g >= 2 * tch)
                    # within-chunk s-tile index per j; valid t-cols = [128r, 512)
                    r = [2 * g + j - 4 * tch for j in range(2)]
                    # pre-zero invalid (above-diagonal) p regions, both heads;
                    # off the exp->PV critical path (runs on gpsimd while the
                    # previous slab computes)
                    pview = p[:].rearrange("p (h j u) -> p h j u", h=2, j=2)
                    if diag:
                        for j in range(2):
                            if r[j] > 0:
                                nc.gpsimd.memset(pview[:, :, j, 0:128 * r[j]], 0.0)
                    for j in range(2):
                        st = 2 * g + j
                        c0 = 128 * r[j] if diag else 0
                        nc.tensor.matmul(
                            sg[:, j * 512 + c0:(j + 1) * 512],
                            qk_sb[0:64, koff + st * 128: koff + st * 128 + 128],
                            qk_sb[0:64, qoff + tch * 512 + c0: qoff + tch * 512 + 512],
                            start=True, stop=True, tile_position=(0, 0),
                        )
                        nc.tensor.matmul(
                            sg[:, 1024 + j * 512 + c0: 1024 + (j + 1) * 512],
                            qk_sb[64:128, koff + st * 128: koff + st * 128 + 128],
                            qk_sb[64:128, qoff + tch * 512 + c0: qoff + tch * 512 + 512],
                            start=True, stop=True, tile_position=(64, 0),
                        )
                    if not diag:
                        nc.scalar.activation(p[:], sg[:], EXP, scale=float(SCALE))
                    else:
                        sgview = sg[:].rearrange("p (h j u) -> p h j u", h=2, j=2)
                        for j in range(2):
                            c0 = 128 * r[j]
                            nc.scalar.activation(pview[:, :, j, c0:512],
                                                 sgview[:, :, j, c0:512],
                                                 EXP, scale=float(SCALE))
                        # triangular boundary block: 0/1 mask on vector (fast)
                        tri = mask_sb[:, 0:128]
                        for j in range(2):
                            c0 = 128 * r[j]
                            for h in range(2):
                                blk = pview[:, h, j, c0:c0 + 128]
                                nc.vector.tensor_mul(blk, blk, tri)
                    # filler matmuls cover the exp latency before PV needs p
                    pull(quota)
                    first, last = (g == 0), (g == n_slab - 1)
                    for j in range(2):
                        st = 2 * g + j
                        nc.tensor.matmul(
                            acc0[:],
                            v_sb[:, st * HPC * VW + (2 * hp) * VW:
                                 st * HPC * VW + (2 * hp) * VW + VW],
                            p[:, j * 512:(j + 1) * 512],
                            start=(first and j == 0), stop=(last and j == 1),
                        )
                        nc.tensor.matmul(
                            acc1[:],
                            v_sb[:, st * HPC * VW + (2 * hp + 1) * VW:
                                 st * HPC * VW + (2 * hp + 1) * VW + VW],
                            p[:, 1024 + j * 512: 1024 + (j + 1) * 512],
                            start=(first and j == 0), stop=(last and j == 1),
                        )
                # normalize: O_norm^T = O^T*(1/l), l on rows 64..127
                for i, acc in ((0, acc0), (1, acc1)):
                    a = 2 * hp + i   # head index in core
                    # full-tile recip: the custom-DVE op mishandles
                    # partition slices; rows 0..63 are garbage, unused
                    rl = rlp.tile([128, 512], F32, tag="rl", name="rl")
                    nc.vector.reciprocal_approx_fast(rl[:], acc[:])
                    po = (a % 2) * 64
                    dst = on_sb[po:po + 64,
                                (a // 2) * T + tch * 512:(a // 2) * T + tch * 512 + 512]
                    nc.vector.tensor_mul(dst, acc[0:D, :], rl[64:128, :])

            # ---- driver: attention phases with qkv/proj interleaved ----
            # per-chunk filler quota (ns per slab): sized so chunk tch's slabs
            # absorb the next chunk's qkv + the previous chunk's proj
            QUOTA = [2800.0, 1900.0, 1250.0, 300.0]
            for tch in range(TCH):
                for hp in range(2):
                    req = set()
                    for c in range(tch + 1):
                        req |= {('k', c, hp), ('q', c, hp)}
                        req |= {('v', c, i) for i in range(4)}
                    drain(req)
                    attention(tch, hp, QUOTA[tch])
                for i in range(4):
                    fifo.append((('c', tch, i), proj_unit(tch, i)))
            while pos[0] < len(fifo):
                step_front()

    nc.compile()
    return nc


def _causal_masks():
    """mask[p, r*512 + j] = 1.0 if (128*r + p) <= j else 0.0, r in 0..3."""
    p = np.arange(128)[:, None]
    j = np.arange(512)[None, :]
    cols = [((128 * r + p) <= j).astype(np.float32) for r in range(4)]
    return np.concatenate(cols, axis=1)


def _in_maps(x, w_qkv, b_qkv, w_proj):
    mask = _causal_masks()
    vones = np.ones((128, NT * HPC * D), dtype=np.float32)
    maps = []
    for core in range(N_CORES):
        b, hg = divmod(core, 4)
        h0 = hg * HPC                       # first global head of this core
        r0 = h0 * D                         # first q row
        q_w = w_qkv[r0:r0 + HPC * D]                    # [256, C]
        k_w = w_qkv[C + r0:C + r0 + HPC * D]
        v_w = w_qkv[2 * C + r0:2 * C + r0 + HPC * D]
        wqkvT = np.ascontiguousarray(np.concatenate([q_w, k_w, v_w], axis=0).T)
        wpT = np.ascontiguousarray(w_proj[:, r0:r0 + HPC * D].T)    # [256, C]
        bqkv = np.ascontiguousarray(np.concatenate(
            [b_qkv[r0:r0 + HPC * D], b_qkv[C + r0:C + r0 + HPC * D]]
        ).reshape(4, 128).T)                                         # [128,4]
        maps.append({
            "xT": np.ascontiguousarray(x[b].T).astype(np.float16),
            "wqkvT": wqkvT.astype(np.float16),
            "wpT": wpT.astype(np.float16),
            "bqkv": bqkv,
            "vones": vones.astype(np.float16),
            "mask": mask.astype(np.float16),
        })
    return maps


def kernel(x, w_qkv, b_qkv, w_proj, b_proj, _trace=False, _tmpdir=None):
    x = np.asarray(x, dtype=np.float32)
    w_qkv = np.asarray(w_qkv, dtype=np.float32)
    b_qkv = np.asarray(b_qkv, dtype=np.float32)
    w_proj = np.asarray(w_proj, dtype=np.float32)
    b_proj = np.asarray(b_proj, dtype=np.float32)

    if "nc" not in _CACHE:
        _CACHE["nc"] = _build()
    nc = _CACHE["nc"]

    maps = _in_maps(x, w_qkv, b_qkv, w_proj)
    kw = {}
    if _trace:
        kw = {"trace": True, "tmpdir": _tmpdir}
    res = run_bass_kernel_spmd(nc, maps, list(range(N_CORES)), **kw)

    # v-bias flows linearly through attention: fold w_proj @ b_v into the
    # output bias added on the host.
    b_eff = w_proj @ b_qkv[2 * C:3 * C] + b_proj
    out = np.empty((B, T, C), dtype=np.float32)
    for b in range(B):
        acc = res.results[4 * b]["y"].astype(np.float32)
        for hg in range(1, 4):
            acc = acc + res.results[4 * b + hg]["y"].astype(np.float32)
        out[b] = acc + b_eff[None, :]
    if _trace:
        return out, res
    return out


# revision 12
# speedup vs baseline: 1.5070x; 1.1267x over previous
"""Causal multi-head attention block (qkv -> attention -> proj) on 8 TRN2 cores.

Problem: x[2,2048,1024], w_qkv[3072,1024], b_qkv[3072], w_proj[1024,1024],
b_proj[1024]; H=16 heads, D=64; softmax scale 1/sqrt(1024).

Sharding: core = (batch b, head-group hg); 2 batches x 4 groups of 4 heads.
Each core computes qkv for its 4 heads, causal attention, and a partial
projection (its heads' columns of w_proj); host sums the 4 partials per batch
and adds the folded bias (w_proj @ b_v + b_proj).

The whole kernel is ONE software-pipelined instruction stream built around
the scalar engine's exp throughput: attention slabs (QK -> exp -> PV) are the
backbone, and qkv / proj matmuls of other t-chunks are interleaved between a
slab's exp and its PV so the tensor engine never stalls waiting on the
scalar engine (which also keeps the PE at its top p-state).

Layouts: everything the PE contracts over is partition-major. q,k are
produced [dims, t] (weights stationary); v is produced directly in [t, dims]
(x tiles stationary) so no PE transposes are needed; the PV stationary
v-tile is [v_h (64) | ones (64)] so the PV matmul yields the softmax
denominator on partitions 64..127 for free. Causality: above-diagonal
s-slabs are skipped; diagonal slabs are masked with precomputed 0/1 masks on
the (otherwise idle) gpsimd engine. QK packs two heads into the PE via row
tiling (K=64), and each slab's scores for both heads live in one
[128,2048] PSUM tile so a single activation instruction exps them all.

All attention-path and projection tensors are fp16 (PSUM accumulation fp32);
the per-core partial y is returned fp16 and summed on the host in fp32.
"""

import numpy as np
from contextlib import ExitStack

import concourse.bass as bass
import concourse.bacc as bacc
import concourse.tile as tile
import concourse.mybir as mybir
from concourse.bass_utils import run_bass_kernel_spmd

B, T, C, H = 2, 2048, 1024, 16
D = C // H                  # 64, head dim
HPC = 4                     # heads per core
N_CORES = 8
NT = T // 128               # 16 t-tiles / s-tiles of 128
NCT = C // 128              # 8 contraction tiles over C
TCH = T // 512              # 4 t-chunks of 512
SCALE = 1.0 / np.sqrt(np.float32(C))   # 1/32

F32 = mybir.dt.float32
F16 = mybir.dt.float16
F8 = mybir.dt.float8e4
EXP = mybir.ActivationFunctionType.Exp
DR = mybir.MatmulPerfMode.DoubleRow

VW = 2 * D                  # 128: per-head block in v_sb = [v_h (64) | ones (64)]
MM_NS = 230.0               # planning est: one N=512 fp16 matmul slot
W8 = 32.0                   # fp8 pre-scale on w_q/w_k (and b_q/b_k)

_CACHE = {}


def _build():
    """Build + compile the SPMD program (identical on all 8 cores)."""
    nc = bacc.Bacc("TRN2", target_bir_lowering=False, debug=False)

    xT = nc.dram_tensor("xT", [C, T], F16, kind="ExternalInput")          # x[b].T
    xT8 = nc.dram_tensor("xT8", [C, T], F8, kind="ExternalInput")         # x[b].T fp8
    wqk8 = nc.dram_tensor("wqk8", [C, 2 * HPC * D], F8, kind="ExternalInput")  # 32*(wq|wk)
    wvT = nc.dram_tensor("wvT", [C, HPC * D], F16, kind="ExternalInput")
    wpT = nc.dram_tensor("wpT", [HPC * D, C], F16, kind="ExternalInput")
    bqkv = nc.dram_tensor("bqkv", [128, 4], F32, kind="ExternalInput")    # 32*(bq|bk)
    vones = nc.dram_tensor("vones", [128, NT * HPC * D], F16, kind="ExternalInput")
    mask = nc.dram_tensor("mask", [128, 2048], F16, kind="ExternalInput")  # 4x[128,512]
    y = nc.dram_tensor("y", [T, C], F16, kind="ExternalOutput")

    with tile.TileContext(nc) as tc, ExitStack() as ctx:
        sb = ctx.enter_context(tc.tile_pool(name="persist", bufs=1))

        # ---- persistent SBUF tensors ----
        wqk8_sb = sb.tile([128, NCT * 512], F8, tag="wqk8")        # [c-tile][m 512]
        wv_sb = sb.tile([128, NCT * 256], F16, tag="wv")           # [c-tile][m 256]
        wp_sb = sb.tile([128, 2 * C], F16, tag="wp")               # [ci-tile][co 1024]
        bqkv_sb = sb.tile([128, 4], F32, tag="bqkv")
        mask_sb = sb.tile([128, 2048], F16, tag="mask")
        x_sb = sb.tile([128, NCT * T], F16, tag="x")               # [c-tile][t]
        x8_sb = sb.tile([128, NCT * T], F8, tag="x8")              # [c-tile][t] fp8
        qk_sb = sb.tile([128, 4 * T], F16, tag="qk")   # q^T|k^T [m-tile][t], 32x scale
        v_sb = sb.tile([128, NT * HPC * VW], F16, tag="v")  # [s-tile][h][v|ones]
        on_sb = sb.tile([128, 2 * T], F16, tag="onorm")     # O_norm^T [ci-tile][t]

        with tc.tile_pool(name="psg", bufs=1, space="PSUM") as psg, \
             tc.tile_pool(name="psacc", bufs=1, space="PSUM") as psacc, \
             tc.tile_pool(name="gem", bufs=2, space="PSUM") as gem, \
             tc.tile_pool(name="att", bufs=4) as att, \
             tc.tile_pool(name="rlp", bufs=2) as rlp, \
             tc.tile_pool(name="yst", bufs=4) as yst:

            # ---- input DMAs (gpsimd queue: cheap dispatch, overlaps all) ----
            for ct in range(NCT):       # q/k fp8 inputs first: smallest prologue
                nc.gpsimd.dma_start(wqk8_sb[:, ct * 512:(ct + 1) * 512],
                                    wqk8.ap()[ct * 128:(ct + 1) * 128, :])
                nc.gpsimd.dma_start(x8_sb[:, ct * T: ct * T + 512],
                                    xT8.ap()[ct * 128:(ct + 1) * 128, 0:512])
            nc.gpsimd.dma_start(bqkv_sb[:], bqkv.ap())
            nc.gpsimd.dma_start(mask_sb[:], mask.ap())
            for ct in range(NCT):       # chunk-0 x fp16 + v weights (for v GEMMs)
                nc.gpsimd.dma_start(wv_sb[:, ct * 256:(ct + 1) * 256],
                                    wvT.ap()[ct * 128:(ct + 1) * 128, :])
                nc.gpsimd.dma_start(x_sb[:, ct * T: ct * T + 512],
                                    xT.ap()[ct * 128:(ct + 1) * 128, 0:512])
            # ones columns of v_sb (softmax denominator trick), cols 64..127/head
            vdst = v_sb[:].rearrange("p (s h e) -> p s h e", s=NT, h=HPC)[:, :, :, D:VW]
            vsrc = vones.ap().rearrange("p (s h e) -> p s h e", s=NT, h=HPC)
            nc.gpsimd.dma_start(vdst, vsrc)
            for ct in range(NCT):       # x chunks 1..3, fp8 then fp16
                nc.gpsimd.dma_start(x8_sb[:, ct * T + 512:(ct + 1) * T],
                                    xT8.ap()[ct * 128:(ct + 1) * 128, 512:T])
            for ct in range(NCT):
                nc.gpsimd.dma_start(x_sb[:, ct * T + 512:(ct + 1) * T],
                                    xT.ap()[ct * 128:(ct + 1) * 128, 512:T])
            for kt in range(2):
                nc.gpsimd.dma_start(wp_sb[:, kt * C:(kt + 1) * C],
                                    wpT.ap()[kt * 128:(kt + 1) * 128, :])

            x8v = x8_sb[:].rearrange("p (c t) -> p c t", c=NCT)
            w8v = wqk8_sb[:].rearrange("p (c m) -> p c m", c=NCT)

            # ---- filler work units (generators; yield ~est ns per PE slot) ----
            def qk_gemm_unit(tch, mt):
                """q or k m-tile GEMM (fp8 DoubleRow) + bias move to qk_sb."""
                acc = gem.tile([128, 512], F32, tag="gacc", name="gacc")
                for cp in range(NCT // 2):
                    nc.tensor.matmul(
                        acc[:],
                        w8v[:, 2 * cp:2 * cp + 2, mt * 128:(mt + 1) * 128],
                        x8v[:, 2 * cp:2 * cp + 2, tch * 512:tch * 512 + 512],
                        start=(cp == 0), stop=(cp == NCT // 2 - 1),
                        perf_mode=DR,
                    )
                    yield MM_NS
                nc.vector.tensor_scalar_add(
                    qk_sb[:, mt * T + tch * 512: mt * T + tch * 512 + 512],
                    acc[:], bqkv_sb[:, mt:mt + 1],
                )

            def v_gemm_unit(tch, i):
                """v for t-tile 4*tch+i, produced directly in [t, m] layout."""
                tt = 4 * tch + i
                acc = gem.tile([128, 512], F32, tag="gacc", name="vacc")
                for ct in range(NCT):
                    nc.tensor.matmul(
                        acc[:, 0:256],
                        x_sb[:, ct * T + tt * 128: ct * T + tt * 128 + 128],
                        wv_sb[:, ct * 256:(ct + 1) * 256],
                        start=(ct == 0), stop=(ct == NCT - 1),
                    )
                    yield MM_NS / 2
                dst = v_sb[:, tt * HPC * VW:(tt + 1) * HPC * VW].rearrange(
                    "p (h e) -> p h e", h=HPC)[:, :, 0:D]
                src = acc[:, 0:256].rearrange("p (h d) -> p h d", h=HPC)
                nc.vector.tensor_copy(dst, src)   # gpsimd can't read PSUM

            def proj_unit(tch, i):
                """proj for t-tile 4*tch+i (all 4 heads), y write-out."""
                tt = 4 * tch + i
                yt = yst.tile([128, C], F16, tag="ytile", name="yt")
                for cc in range(2):
                    acc = gem.tile([128, 512], F32, tag="gacc", name="pacc")
                    for kt in range(2):
                        nc.tensor.matmul(
                            acc[:],
                            on_sb[:, kt * T + tt * 128: kt * T + tt * 128 + 128],
                            wp_sb[:, kt * C + cc * 512: kt * C + cc * 512 + 512],
                            start=(kt == 0), stop=(kt == 1),
                        )
                        yield MM_NS
                    nc.vector.tensor_copy(yt[:, cc * 512:(cc + 1) * 512], acc[:])
                nc.sync.dma_start(y.ap()[tt * 128:(tt + 1) * 128, :], yt[:])

            # FIFO of filler units; per chunk: hp0's k,q first, then v, then hp1
            fifo = []
            for tch in range(TCH):
                fifo.append((('k', tch, 0), qk_gemm_unit(tch, 2)))
                fifo.append((('q', tch, 0), qk_gemm_unit(tch, 0)))
                for i in range(4):
                    fifo.append((('v', tch, i), v_gemm_unit(tch, i)))
                fifo.append((('k', tch, 1), qk_gemm_unit(tch, 3)))
                fifo.append((('q', tch, 1), qk_gemm_unit(tch, 1)))

            pos = [0]
            done = set()

            def step_front():
                """Advance the head unit by one PE slot; returns est ns."""
                tag, gen = fifo[pos[0]]
                try:
                    return next(gen)
                except StopIteration:
                    done.add(tag)
                    pos[0] += 1
                    return 0.0

            def pull(ns):
                spent = 0.0
                while spent < ns and pos[0] < len(fifo):
                    spent += step_front()

            def drain(tags):
                while not tags.issubset(done) and pos[0] < len(fifo):
                    step_front()

            # ---- attention backbone: slabs of 2 s-tiles x 512 t, 2 heads ----
            SCL = float(SCALE / (W8 * W8))   # q,k carry the fp8 32x pre-scale

            def attention(tch, hp, quota):
                qoff = hp * T        # q m-tile = hp
                koff = (2 + hp) * T  # k m-tile = 2+hp
                acc0 = psacc.tile([128, 512], F32, tag="acc0", name="acc0")
                acc1 = psacc.tile([128, 512], F32, tag="acc1", name="acc1")
                n_slab = 2 * (tch + 1)

                def qk_slab(g):
                    """Scores + exp + causal handling for slab g; returns p."""
                    sg = psg.tile([128, 2048], F32, tag="sG", name="sg")
                    p = att.tile([128, 2048], F16, tag="p", name="p")
                    diag = (g >= 2 * tch)
                    # within-chunk s-tile index per j; valid t-cols = [128r, 512)
                    r = [2 * g + j - 4 * tch for j in range(2)]
                    pview = p[:].rearrange("p (h j u) -> p h j u", h=2, j=2)
                    if diag:
                        # pre-zero invalid (above-diagonal) p; off critical path
                        for j in range(2):
                            if r[j] > 0:
                                nc.gpsimd.memset(pview[:, :, j, 0:128 * r[j]], 0.0)
                    for j in range(2):
                        st = 2 * g + j
                        c0 = 128 * r[j] if diag else 0
                        nc.tensor.matmul(
                            sg[:, j * 512 + c0:(j + 1) * 512],
                            qk_sb[0:64, koff + st * 128: koff + st * 128 + 128],
                            qk_sb[0:64, qoff + tch * 512 + c0: qoff + tch * 512 + 512],
                            start=True, stop=True, tile_position=(0, 0),
                        )
                        nc.tensor.matmul(
                            sg[:, 1024 + j * 512 + c0: 1024 + (j + 1) * 512],
                            qk_sb[64:128, koff + st * 128: koff + st * 128 + 128],
                            qk_sb[64:128, qoff + tch * 512 + c0: qoff + tch * 512 + 512],
                            start=True, stop=True, tile_position=(64, 0),
                        )
                    if not diag:
                        nc.scalar.activation(p[:], sg[:], EXP, scale=SCL)
                    else:
                        sgview = sg[:].rearrange("p (h j u) -> p h j u", h=2, j=2)
                        for j in range(2):
                            c0 = 128 * r[j]
                            nc.scalar.activation(pview[:, :, j, c0:512],
                                                 sgview[:, :, j, c0:512],
                                                 EXP, scale=SCL)
                        # triangular boundary block: 0/1 mask on vector (fast)
                        tri = mask_sb[:, 0:128]
                        for j in range(2):
                            c0 = 128 * r[j]
                            for h in range(2):
                                blk = pview[:, h, j, c0:c0 + 128]
                                nc.vector.tensor_mul(blk, blk, tri)
                    return p

                def pv_slab(g, p):
                    first, last = (g == 0), (g == n_slab - 1)
                    for j in range(2):
                        st = 2 * g + j
                        nc.tensor.matmul(
                            acc0[:],
                            v_sb[:, st * HPC * VW + (2 * hp) * VW:
                                 st * HPC * VW + (2 * hp) * VW + VW],
                            p[:, j * 512:(j + 1) * 512],
                            start=(first and j == 0), stop=(last and j == 1),
                        )
                        nc.tensor.matmul(
                            acc1[:],
                            v_sb[:, st * HPC * VW + (2 * hp + 1) * VW:
                                 st * HPC * VW + (2 * hp + 1) * VW + VW],
                            p[:, 1024 + j * 512: 1024 + (j + 1) * 512],
                            start=(first and j == 0), stop=(last and j == 1),
                        )

                # software pipeline: QK(g+1) issues before PV(g) so the scalar
                # engine's exp stream is only ever one QK behind the tensor
                # engine; fillers absorb the remaining exp latency
                p_cur = qk_slab(0)
                for g in range(n_slab):
                    pull(quota)
                    p_next = qk_slab(g + 1) if g + 1 < n_slab else None
                    pv_slab(g, p_cur)
                    p_cur = p_next
                # normalize: O_norm^T = O^T*(1/l), l on rows 64..127
                for i, acc in ((0, acc0), (1, acc1)):
                    a = 2 * hp + i   # head index in core
                    # full-tile recip: the custom-DVE op mishandles
                    # partition slices; rows 0..63 are garbage, unused
                    rl = rlp.tile([128, 512], F32, tag="rl", name="rl")
                    nc.vector.reciprocal_approx_fast(rl[:], acc[:])
                    po = (a % 2) * 64
                    dst = on_sb[po:po + 64,
                                (a // 2) * T + tch * 512:(a // 2) * T + tch * 512 + 512]
                    nc.vector.tensor_mul(dst, acc[0:D, :], rl[64:128, :])

            # ---- driver: attention phases with qkv/proj interleaved ----
            # per-chunk filler quota (ns per slab): sized so chunk tch's slabs
            # absorb the next chunk's qkv + the previous chunk's proj
            QUOTA = [1700.0, 1300.0, 900.0, 260.0]
            for tch in range(TCH):
                for hp in range(2):
                    req = set()
                    for c in range(tch + 1):
                        req |= {('k', c, hp), ('q', c, hp)}
                        req |= {('v', c, i) for i in range(4)}
                    drain(req)
                    attention(tch, hp, QUOTA[tch])
                for i in range(4):
                    fifo.append((('c', tch, i), proj_unit(tch, i)))
            while pos[0] < len(fifo):
                step_front()

    nc.compile()
    return nc


def _causal_masks():
    """mask[p, r*512 + j] = 1.0 if (128*r + p) <= j else 0.0, r in 0..3."""
    p = np.arange(128)[:, None]
    j = np.arange(512)[None, :]
    cols = [((128 * r + p) <= j).astype(np.float32) for r in range(4)]
    return np.concatenate(cols, axis=1)


def _in_maps(x, w_qkv, b_qkv, w_proj):
    F8NP = mybir.dt.np(F8)
    mask = _causal_masks()
    vones = np.ones((128, NT * HPC * D), dtype=np.float32)
    xT16 = [np.ascontiguousarray(x[b].T).astype(np.float16) for b in range(B)]
    xT8 = [np.ascontiguousarray(x[b].T).astype(F8NP) for b in range(B)]
    maps = []
    for core in range(N_CORES):
        b, hg = divmod(core, 4)
        h0 = hg * HPC                       # first global head of this core
        r0 = h0 * D                         # first q row
        q_w = w_qkv[r0:r0 + HPC * D]                    # [256, C]
        k_w = w_qkv[C + r0:C + r0 + HPC * D]
        v_w = w_qkv[2 * C + r0:2 * C + r0 + HPC * D]
        wqk8 = np.ascontiguousarray(
            (np.concatenate([q_w, k_w], axis=0) * W8).T).astype(F8NP)
        wvT = np.ascontiguousarray(v_w.T)                           # [C, 256]
        wpT = np.ascontiguousarray(w_proj[:, r0:r0 + HPC * D].T)    # [256, C]
        bqkv = np.ascontiguousarray(np.concatenate(
            [b_qkv[r0:r0 + HPC * D], b_qkv[C + r0:C + r0 + HPC * D]]
        ).reshape(4, 128).T) * W8                                    # [128,4]
        maps.append({
            "xT": xT16[b],
            "xT8": xT8[b],
            "wqk8": wqk8,
            "wvT": wvT.astype(np.float16),
            "wpT": wpT.astype(np.float16),
            "bqkv": np.ascontiguousarray(bqkv),
            "vones": vones.astype(np.float16),
            "mask": mask.astype(np.float16),
        })
    return maps


def kernel(x, w_qkv, b_qkv, w_proj, b_proj, _trace=False, _tmpdir=None):
    x = np.asarray(x, dtype=np.float32)
    w_qkv = np.asarray(w_qkv, dtype=np.float32)
    b_qkv = np.asarray(b_qkv, dtype=np.float32)
    w_proj = np.asarray(w_proj, dtype=np.float32)
    b_proj = np.asarray(b_proj, dtype=np.float32)

    if "nc" not in _CACHE:
        _CACHE["nc"] = _build()
    nc = _CACHE["nc"]

    maps = _in_maps(x, w_qkv, b_qkv, w_proj)
    kw = {}
    if _trace:
        kw = {"trace": True, "tmpdir": _tmpdir}
    res = run_bass_kernel_spmd(nc, maps, list(range(N_CORES)), **kw)

    # v-bias flows linearly through attention: fold w_proj @ b_v into the
    # output bias added on the host.
    b_eff = w_proj @ b_qkv[2 * C:3 * C] + b_proj
    out = np.empty((B, T, C), dtype=np.float32)
    for b in range(B):
        acc = res.results[4 * b]["y"].astype(np.float32)
        for hg in range(1, 4):
            acc = acc + res.results[4 * b + hg]["y"].astype(np.float32)
        out[b] = acc + b_eff[None, :]
    if _trace:
        return out, res
    return out
